# revision 15
# baseline (speedup 1.0000x reference)
"""Deformable group-correlation kernel for TRN2 (8 NeuronCores).

Reference op: bilinear-sample right_feature at per-pixel coords
(base grid + flow + 1x9 window offsets + extra offsets), then group-wise
(4 groups x 64ch) mean of left * sampled -> (2, 36, 80, 160).

Sharding: 8 cores = (batch b in {0,1}) x (group g in {0..3}).
Each core handles its (b, g): 64 channels, full 80x160 spatial.

Per-core pipeline (20 blocks of 4 pixel rows, 5760 samples each):
  - host: zero-padded channel-last 4-corner patch table r4[(y,x), 4*64] bf16
  - DVE: coords -> int32 granule row offsets + bilinear corner weights
  - gpsimd indirect_dma_start (hardware dynamic DMA): gathers one 512B
    granule per sample into patch[sample_partition, slot, 4*64]
  - DVE: prodW = patch * left_view ; tensor_reduce X (64ch) -> t[p,slot,4]
  - DVE: corr = sum_k w_k * t_k  -> out DMA
"""

import sys

sys.path.insert(0, "/opt/trn_rl_repo")

from contextlib import ExitStack

import numpy as np
import ml_dtypes

from concourse import bacc, bass, mybir
from concourse.bass_utils import run_bass_kernel_spmd

F32 = mybir.dt.float32
BF16 = mybir.dt.bfloat16
I32 = mybir.dt.int32
AF = mybir.AluOpType

B, C, H, W = 2, 256, 80, 160
G, gC, S = 4, 64, 9
PADDING = 2
TAB_H, TAB_W = 84, 164  # table: y in [0,84), x in [0,164); row = y*164 + x
NROWS = TAB_H * TAB_W  # 13776 granule rows
ELEM = 4 * gC  # 256 bf16 per granule = 512B
NBLK = 20  # blocks of 4 pixel rows
PIX = 4 * W  # 640 pixels per block
JJ = 5  # 640 = 5 * 128
NSLOT = S * JJ  # 45 slots of 128 samples per block
NIDX = S * PIX  # 5760 samples per block
NCO = NBLK * NSLOT  # 900 coords per partition (all blocks)
MAGIC = 8388608.0  # 2**23

_graph_cache = {}


def _build_graph():
    nc = bacc.Bacc("TRN2")

    r4 = nc.declare_dram_parameter("r4", [NROWS, ELEM], BF16, isOutput=False)
    lt = nc.declare_dram_parameter("lt", [128, NBLK * JJ * gC], BF16, isOutput=False)
    cb = nc.declare_dram_parameter("cb", [128, NBLK * JJ * 2], F32, isOutput=False)
    ex = nc.declare_dram_parameter("ex", [128, NCO * 2], F32, isOutput=False)
    out = nc.declare_dram_parameter("out", [NBLK * 128, NSLOT], F32, isOutput=True)

    with ExitStack() as stk:
        sb = lambda name, shape, dt: stk.enter_context(nc.sbuf_tensor(name, shape, dt))
        lt_s = sb("lt_s", [128, NBLK * JJ * gC], BF16)
        cb_s = sb("cb_s", [128, NBLK * JJ * 2], F32)
        ex_s = sb("ex_s", [128, NCO * 2], F32)
        u0 = sb("u0", [128, NCO], F32)
        u1 = sb("u1", [128, NCO], F32)
        u2 = sb("u2", [128, NCO], F32)
        u3 = sb("u3", [128, NCO], F32)
        u4 = sb("u4", [128, NCO], F32)
        wts = sb("wts", [128, NCO * 4], BF16)
        idx = sb("idx", [128, NCO], I32)
        patch0 = sb("patch0", [128, NSLOT * ELEM], BF16)
        patch1 = sb("patch1", [128, NSLOT * ELEM], BF16)
        prodw = sb("prodw", [128, NSLOT * ELEM], BF16)
        tbuf = sb("tbuf", [128, NSLOT * 4], BF16)
        mbuf = sb("mbuf", [128, NSLOT * 4], BF16)
        corr0 = sb("corr0", [128, NSLOT], F32)
        corr1 = sb("corr1", [128, NSLOT], F32)
        sem = lambda name: stk.enter_context(nc.semaphore(name))
        load_sem = sem("load_sem")
        coord_sem = sem("coord_sem")
        gat_sems = [sem("gat_sem0"), sem("gat_sem1")]
        prod_sem = sem("prod_sem")
        blend_sem = sem("blend_sem")
        out_sems = [sem("out_sem0"), sem("out_sem1")]
        patches = [patch0, patch1]
        corrs = [corr0, corr1]

        with nc.Block() as block:

            @block.sync
            def _(sync):
                sync.dma_start(lt_s[:, :], lt[:, :]).then_inc(load_sem, 16)
                sync.dma_start(cb_s[:, :], cb[:, :]).then_inc(load_sem, 16)
                sync.dma_start(ex_s[:, :], ex[:, :]).then_inc(load_sem, 16)

            @block.vector
            def _(vector):
                vector.wait_ge(load_sem, 48)

                # coord views, iteration order (blk, s, j):
                def cbv(comp):
                    return bass.AP(
                        cb_s,
                        comp,
                        [[NBLK * JJ * 2, 128], [JJ * 2, NBLK], [0, S], [2, JJ]],
                    )

                def exv(comp):
                    return bass.AP(
                        ex_s,
                        comp,
                        [[NCO * 2, 128], [NSLOT * 2, NBLK], [JJ * 2, S], [2, JJ]],
                    )

                def co3(t):  # [128,(blk,s,j)] contiguous view of a [128,NCO] buf
                    return bass.AP(
                        t, 0, [[NCO, 128], [NSLOT, NBLK], [JJ, S], [1, JJ]]
                    )

                def flat(t):
                    return t[:, :]

                # x: u0 = cb_x + ex_x (clamped); u1 = round(u0-.5); u2 = frac
                vector.tensor_tensor(out=co3(u0), in0=exv(0), in1=cbv(0), op=AF.add)
                vector.drain()
                vector.tensor_scalar_max(flat(u0), flat(u0), 0.5)
                vector.drain()
                vector.tensor_scalar_min(flat(u0), flat(u0), TAB_W - 1.5)
                vector.drain()
                vector.tensor_scalar_add(flat(u1), flat(u0), MAGIC - 0.5)
                vector.drain()
                vector.tensor_scalar_add(flat(u1), flat(u1), -MAGIC)
                vector.drain()
                vector.tensor_tensor(
                    out=flat(u2), in0=flat(u0), in1=flat(u1), op=AF.subtract
                )
                vector.drain()
                # y: u0 = cb_y + ex_y (clamped); u3 = round(u0-.5); u4 = frac
                vector.tensor_tensor(out=co3(u0), in0=exv(1), in1=cbv(1), op=AF.add)
                vector.drain()
                vector.tensor_scalar_max(flat(u0), flat(u0), 0.5)
                vector.drain()
                vector.tensor_scalar_min(flat(u0), flat(u0), TAB_H - 1.5)
                vector.drain()
                vector.tensor_scalar_add(flat(u3), flat(u0), MAGIC - 0.5)
                vector.drain()
                vector.tensor_scalar_add(flat(u3), flat(u3), -MAGIC)
                vector.drain()
                vector.tensor_tensor(
                    out=flat(u4), in0=flat(u0), in1=flat(u3), op=AF.subtract
                )
                vector.drain()
                # row = y0f * TAB_W + x0f -> int32 idx
                vector.scalar_tensor_tensor(
                    out=flat(u0),
                    in0=flat(u3),
                    scalar=float(TAB_W),
                    in1=flat(u1),
                    op0=AF.mult,
                    op1=AF.add,
                )
                vector.drain()
                vector.tensor_scalar_add(idx[:, :], flat(u0), 0.0).then_inc(
                    coord_sem, 1
                )
                vector.drain()
                # weights: u1 = 1-fx, u3 = 1-fy (x0f/y0f no longer needed)
                vector.tensor_scalar(
                    out=flat(u1),
                    in0=flat(u2),
                    scalar1=-1.0,
                    op0=AF.mult,
                    scalar2=1.0,
                    op1=AF.add,
                )
                vector.drain()
                vector.tensor_scalar(
                    out=flat(u3),
                    in0=flat(u4),
                    scalar1=-1.0,
                    op0=AF.mult,
                    scalar2=1.0,
                    op1=AF.add,
                )
                vector.drain()

                def wv(k):
                    return bass.AP(wts, k, [[NCO * 4, 128], [4, NCO]])

                vector.tensor_tensor(out=wv(0), in0=flat(u1), in1=flat(u3), op=AF.mult)
                vector.tensor_tensor(out=wv(1), in0=flat(u2), in1=flat(u3), op=AF.mult)
                vector.tensor_tensor(out=wv(2), in0=flat(u1), in1=flat(u4), op=AF.mult)
                vector.tensor_tensor(out=wv(3), in0=flat(u2), in1=flat(u4), op=AF.mult)
                vector.drain()

                def doblock(n):
                    pb = patches[n % 2]
                    vector.wait_ge(gat_sems[n % 2], 16 * (n // 2 + 1))
                    # prodW[p,(s,j,k,c)] = patch * left ; split per s (3 free dims max)
                    for s in range(S):
                        in0 = bass.AP(
                            pb,
                            s * JJ * ELEM,
                            [[NSLOT * ELEM, 128], [ELEM, JJ], [gC, 4], [1, gC]],
                        )
                        in1 = bass.AP(
                            lt_s,
                            n * JJ * gC,
                            [[NBLK * JJ * gC, 128], [gC, JJ], [0, 4], [1, gC]],
                        )
                        o = bass.AP(
                            prodw,
                            s * JJ * ELEM,
                            [[NSLOT * ELEM, 128], [ELEM, JJ], [gC, 4], [1, gC]],
                        )
                        mm = vector.tensor_tensor(out=o, in0=in0, in1=in1, op=AF.mult)
                    mm.then_inc(prod_sem, 1)
                    vector.drain()
                    # t[p, slot, k] = sum_c prodw
                    pin = bass.AP(
                        prodw, 0, [[NSLOT * ELEM, 128], [gC, NSLOT * 4], [1, gC]]
                    )
                    tv = bass.AP(tbuf, 0, [[NSLOT * 4, 128], [1, NSLOT * 4]])
                    with nc.allow_low_precision(reason="bf16 64-ch dot accumulate"):
                        vector.tensor_reduce(
                            out=tv, in_=pin, axis=mybir.AxisListType.X, op=AF.add
                        )
                    vector.drain()
                    # corr[p, slot] = sum_k t*w
                    if n >= 2:
                        vector.wait_ge(out_sems[n % 2], 16 * ((n - 2) // 2 + 1))
                    m = bass.AP(mbuf, 0, [[NSLOT * 4, 128], [1, NSLOT * 4]])
                    wv2 = bass.AP(
                        wts, n * NSLOT * 4, [[NCO * 4, 128], [1, NSLOT * 4]]
                    )
                    vector.tensor_tensor(out=m, in0=tv, in1=wv2, op=AF.mult)
                    vector.drain()
                    m3 = bass.AP(mbuf, 0, [[NSLOT * 4, 128], [4, NSLOT], [1, 4]])
                    cv = bass.AP(corrs[n % 2], 0, [[NSLOT, 128], [1, NSLOT]])
                    vector.tensor_reduce(
                        out=cv, in_=m3, axis=mybir.AxisListType.X, op=AF.add
                    ).then_inc(blend_sem, 1)
                    vector.drain()

                for n in range(NBLK):
                    doblock(n)

            @block.gpsimd
            def _(gpsimd):
                gpsimd.wait_ge(coord_sem, 1)
                for n in range(NBLK):
                    if n >= 2:
                        gpsimd.wait_ge(prod_sem, n - 1)
                    pb = patches[n % 2]
                    dst = bass.AP(
                        pb, 0, [[NSLOT * ELEM, 128], [ELEM, NSLOT], [1, ELEM]]
                    )
                    off = bass.AP(idx, n * NSLOT, [[NCO, 128], [1, NSLOT]])
                    gpsimd.indirect_dma_start(
                        out=dst,
                        out_offset=None,
                        in_=r4[:, :],
                        in_offset=bass.IndirectOffsetOnAxis(ap=off, axis=0),
                    ).then_inc(gat_sems[n % 2], 16)

            @block.scalar
            def _(scalar):
                for n in range(NBLK):
                    scalar.wait_ge(blend_sem, n + 1)
                    dst = out[n * 128 : (n + 1) * 128, :]
                    scalar.dma_start(dst, corrs[n % 2][:, :]).then_inc(
                        out_sems[n % 2], 16
                    )
                scalar.wait_ge(out_sems[0], 16 * (NBLK // 2))
                scalar.wait_ge(out_sems[1], 16 * (NBLK // 2))

    if not nc.is_finalized():
        nc.finalize()
    return nc


def _host_prep(left_feature, right_feature, flow, extra_offset):
    """Build per-core input dicts. Core ordering: core = b*4 + g."""
    lf = np.asarray(left_feature, np.float32)
    rf = np.asarray(right_feature, np.float32)
    fl = np.asarray(flow, np.float32)
    eo = np.asarray(extra_offset, np.float32)

    # sample (s, j, p) in block blk: pixel pi = j*128+p; (q, w) = divmod(pi, W)
    p_idx = np.arange(128)
    j_idx = np.arange(JJ)
    pi = j_idx[None, :] * 128 + p_idx[:, None]  # [128, 5]
    q = pi // W
    w = pi % W
    blk = np.arange(NBLK)
    h = blk[:, None, None] * 4 + q[None]  # [20, 128, 5]
    wfull = np.broadcast_to(w[None], (NBLK, 128, JJ))

    offx = np.arange(S, dtype=np.float32) - 4.0

    in_maps = []
    for b in range(B):
        fx = fl[b, 0][h, wfull]  # [20,128,5]
        fy = fl[b, 1][h, wfull]
        cbx = wfull.astype(np.float32) + fx + PADDING
        cby = h.astype(np.float32) + fy + PADDING
        cb_np = np.ascontiguousarray(
            np.stack([cbx, cby], -1).transpose(1, 0, 2, 3).reshape(128, -1)
        )

        eo_b = eo[b].reshape(S, 2, H, W)
        exx = eo_b[:, 0][:, h, wfull] + offx[:, None, None, None]  # [S,20,128,5]
        exy = eo_b[:, 1][:, h, wfull]
        ex_np = np.ascontiguousarray(
            np.stack([exx, exy], -1)  # [S,20,128,5,2]
            .transpose(2, 1, 0, 3, 4)  # [128,20,S,5,2]
            .reshape(128, -1)
        )

        for g in range(G):
            csl = slice(g * gC, (g + 1) * gC)
            # lt[p, blk, j, c] = left[c, blk*4 + pi//W, pi%W] / gC, pi=j*128+p
            lslice = lf[b, csl] / gC  # [64, H, W]
            l4 = lslice[:, h, wfull]  # [64, 20, 128, 5]
            lt_np = np.ascontiguousarray(
                l4.transpose(2, 1, 3, 0).reshape(128, -1).astype(ml_dtypes.bfloat16)
            )

            # r4 patch table
            rp = np.zeros((TAB_H + 1, TAB_W + 1, gC), np.float32)
            rp[PADDING : PADDING + H, PADDING : PADDING + W] = rf[b, csl].transpose(
                1, 2, 0
            )
            r4_np = np.ascontiguousarray(
                np.stack(
                    [
                        rp[0:TAB_H, 0:TAB_W],
                        rp[0:TAB_H, 1 : TAB_W + 1],
                        rp[1 : TAB_H + 1, 0:TAB_W],
                        rp[1 : TAB_H + 1, 1 : TAB_W + 1],
                    ],
                    axis=2,
                )
                .reshape(NROWS, ELEM)
                .astype(ml_dtypes.bfloat16)
            )

            in_maps.append(
                {
                    "r4": r4_np,
                    "lt": lt_np,
                    "cb": cb_np,
                    "ex": ex_np,
                }
            )
    return in_maps


def kernel(**inputs):
    if "nc" not in _graph_cache:
        _graph_cache["nc"] = _build_graph()
    nc = _graph_cache["nc"]

    in_maps = _host_prep(
        inputs["left_feature"],
        inputs["right_feature"],
        inputs["flow"],
        inputs["extra_offset"],
    )
    res = run_bass_kernel_spmd(nc, in_maps, core_ids=list(range(8)))
    _graph_cache["last_res"] = res
    outs = [r["out"] for r in res.results]

    full = np.zeros((B, G * S, H, W), np.float32)
    for core in range(8):
        b, g = divmod(core, G)
        o = np.asarray(outs[core], np.float32).reshape(NBLK, 128, S, JJ)
        # [blk, p, s, j] -> [blk, s, (j,p)=pix] -> [blk, s, q, w]
        o = o.transpose(0, 2, 3, 1).reshape(NBLK, S, 4, W)
        for s in range(S):
            full[b, g * S + s] = o[:, s].reshape(H, W)
    return full


# revision 16
# speedup vs baseline: 4.7020x; 4.7020x over previous
"""Deformable group-correlation kernel for TRN2 (8 NeuronCores).

Reference op: bilinear-sample right_feature at per-pixel coords
(base grid + flow + 1x9 window offsets + extra offsets), then group-wise
(4 groups x 64ch) mean of left * sampled -> (2, 36, 80, 160).

Sharding: 8 cores = (batch b in {0,1}) x (group g in {0..3}).
Each core handles its (b, g): 64 channels, full 80x160 spatial.

Per-core pipeline (20 blocks of 4 pixel rows, 5760 samples each):
  - host: zero-padded channel-last 4-corner patch table r4[(y,x), 4*64] bf16
  - DVE: coords -> int32 granule row offsets + bilinear corner weights
  - gpsimd indirect_dma_start (hardware dynamic DMA): gathers one 512B
    granule per sample into patch[sample_partition, slot, 4*64]
  - DVE: prodW = patch * left_view ; tensor_reduce X (64ch) -> t[p,slot,4]
  - DVE: corr = sum_k w_k * t_k  -> out DMA
"""

import sys

sys.path.insert(0, "/opt/trn_rl_repo")

from contextlib import ExitStack

import numpy as np
import ml_dtypes

from concourse import bacc, bass, mybir
from concourse.bass_utils import run_bass_kernel_spmd
from concourse.library_config import mlp as mlp_library

F32 = mybir.dt.float32
BF16 = mybir.dt.bfloat16
I16 = mybir.dt.int16
AF = mybir.AluOpType

B, C, H, W = 2, 256, 80, 160
G, gC, S = 4, 64, 9
PADDING = 2
TAB_H, TAB_W = 84, 164  # table: y in [0,84), x in [0,164); row = y*164 + x
NROWS = TAB_H * TAB_W  # 13776 granule rows
ELEM = 4 * gC  # 256 bf16 per granule = 512B
NBLK = 20  # blocks of 4 pixel rows
PIX = 4 * W  # 640 pixels per block
JJ = 5  # 640 = 5 * 128
NSLOT = S * JJ  # 45 slots of 128 samples per block
NIDX = S * PIX  # 5760 samples per block
NCO = NBLK * NSLOT  # 900 coords per partition (all blocks)
MAGIC = 8388608.0  # 2**23

_graph_cache = {}


def _build_graph():
    nc = bacc.Bacc("TRN2")

    r4 = nc.declare_dram_parameter("r4", [NROWS, ELEM], BF16, isOutput=False)
    lt = nc.declare_dram_parameter("lt", [128, NBLK * JJ * gC], BF16, isOutput=False)
    cb = nc.declare_dram_parameter("cb", [128, NBLK * JJ * 2], F32, isOutput=False)
    ex = nc.declare_dram_parameter("ex", [128, NCO * 2], F32, isOutput=False)
    out = nc.declare_dram_parameter("out", [NBLK * 128, NSLOT], F32, isOutput=True)

    with ExitStack() as stk:
        sb = lambda name, shape, dt: stk.enter_context(nc.sbuf_tensor(name, shape, dt))
        lt_s = sb("lt_s", [128, NBLK * JJ * gC], BF16)
        cb_s = sb("cb_s", [128, NBLK * JJ * 2], F32)
        ex_s = sb("ex_s", [128, NCO * 2], F32)
        u0 = sb("u0", [128, NCO], F32)
        u1 = sb("u1", [128, NCO], F32)
        u2 = sb("u2", [128, NCO], F32)
        u3 = sb("u3", [128, NCO], F32)
        u4 = sb("u4", [128, NCO], F32)
        wts = sb("wts", [128, NCO * 4], BF16)
        idx = sb("idx", [128, NCO], I16)
        wrap = sb("wrap", [128, NBLK * 360], I16)
        patch0 = sb("patch0", [128, NSLOT * ELEM], BF16)
        patch1 = sb("patch1", [128, NSLOT * ELEM], BF16)
        prodw = sb("prodw", [128, NSLOT * ELEM], BF16)
        tbuf = sb("tbuf", [128, NSLOT * 4], BF16)
        mbuf = sb("mbuf", [128, NSLOT * 4], BF16)
        corr0 = sb("corr0", [128, NSLOT], F32)
        corr1 = sb("corr1", [128, NSLOT], F32)
        sem = lambda name: stk.enter_context(nc.semaphore(name))
        load_sem = sem("load_sem")
        coord_sem = sem("coord_sem")
        wrap_sem = sem("wrap_sem")
        gat_sems = [sem("gat_sem0"), sem("gat_sem1")]
        prod_sem = sem("prod_sem")
        blend_sem = sem("blend_sem")
        out_sems = [sem("out_sem0"), sem("out_sem1")]
        patches = [patch0, patch1]
        corrs = [corr0, corr1]

        with nc.Block() as block:

            @block.sync
            def _(sync):
                sync.dma_start(lt_s[:, :], lt[:, :]).then_inc(load_sem, 16)
                sync.dma_start(cb_s[:, :], cb[:, :]).then_inc(load_sem, 16)
                sync.dma_start(ex_s[:, :], ex[:, :]).then_inc(load_sem, 16)

                # index wrap: fold [128, (blk,sj)] -> [16, (blk, sj*8 + a)]
                sync.wait_ge(coord_sem, 1)
                with nc.allow_non_contiguous_dma(
                    reason="one-time 16-wrap index fold, 2B elements"
                ):
                    for a in range(8):
                        srcap = bass.AP(
                            idx, a * 16 * NCO, [[NCO, 16], [NSLOT, NBLK], [1, NSLOT]]
                        )
                        dstap = bass.AP(
                            wrap, a, [[NBLK * 360, 16], [360, NBLK], [8, NSLOT]]
                        )
                        sync.dma_start(dstap, srcap).then_inc(wrap_sem, 16)
                sync.wait_ge(wrap_sem, 128)
                for r in range(1, 8):
                    sync.dma_start(
                        wrap[16 * r : 16 * (r + 1), :], wrap[0:16, :]
                    ).then_inc(wrap_sem, 16)

            @block.vector
            def _(vector):
                vector.wait_ge(load_sem, 48)

                # coord views, iteration order (blk, s, j):
                def cbv(comp):
                    return bass.AP(
                        cb_s,
                        comp,
                        [[NBLK * JJ * 2, 128], [JJ * 2, NBLK], [0, S], [2, JJ]],
                    )

                def exv(comp):
                    return bass.AP(
                        ex_s,
                        comp,
                        [[NCO * 2, 128], [NSLOT * 2, NBLK], [JJ * 2, S], [2, JJ]],
                    )

                def co3(t):  # [128,(blk,s,j)] contiguous view of a [128,NCO] buf
                    return bass.AP(
                        t, 0, [[NCO, 128], [NSLOT, NBLK], [JJ, S], [1, JJ]]
                    )

                def flat(t):
                    return t[:, :]

                # x: u0 = cb_x + ex_x (clamped); u1 = round(u0-.5); u2 = frac
                vector.tensor_tensor(out=co3(u0), in0=exv(0), in1=cbv(0), op=AF.add)
                vector.drain()
                vector.tensor_scalar_max(flat(u0), flat(u0), 0.5)
                vector.drain()
                vector.tensor_scalar_min(flat(u0), flat(u0), TAB_W - 1.5)
                vector.drain()
                vector.tensor_scalar_add(flat(u1), flat(u0), MAGIC - 0.5)
                vector.drain()
                vector.tensor_scalar_add(flat(u1), flat(u1), -MAGIC)
                vector.drain()
                vector.tensor_tensor(
                    out=flat(u2), in0=flat(u0), in1=flat(u1), op=AF.subtract
                )
                vector.drain()
                # y: u0 = cb_y + ex_y (clamped); u3 = round(u0-.5); u4 = frac
                vector.tensor_tensor(out=co3(u0), in0=exv(1), in1=cbv(1), op=AF.add)
                vector.drain()
                vector.tensor_scalar_max(flat(u0), flat(u0), 0.5)
                vector.drain()
                vector.tensor_scalar_min(flat(u0), flat(u0), TAB_H - 1.5)
                vector.drain()
                vector.tensor_scalar_add(flat(u3), flat(u0), MAGIC - 0.5)
                vector.drain()
                vector.tensor_scalar_add(flat(u3), flat(u3), -MAGIC)
                vector.drain()
                vector.tensor_tensor(
                    out=flat(u4), in0=flat(u0), in1=flat(u3), op=AF.subtract
                )
                vector.drain()
                # row = y0f * TAB_W + x0f -> int32 idx
                vector.scalar_tensor_tensor(
                    out=flat(u0),
                    in0=flat(u3),
                    scalar=float(TAB_W),
                    in1=flat(u1),
                    op0=AF.mult,
                    op1=AF.add,
                )
                vector.drain()
                vector.tensor_scalar_add(idx[:, :], flat(u0), 0.0).then_inc(
                    coord_sem, 1
                )
                vector.drain()
                # weights: u1 = 1-fx, u3 = 1-fy (x0f/y0f no longer needed)
                vector.tensor_scalar(
                    out=flat(u1),
                    in0=flat(u2),
                    scalar1=-1.0,
                    op0=AF.mult,
                    scalar2=1.0,
                    op1=AF.add,
                )
                vector.drain()
                vector.tensor_scalar(
                    out=flat(u3),
                    in0=flat(u4),
                    scalar1=-1.0,
                    op0=AF.mult,
                    scalar2=1.0,
                    op1=AF.add,
                )
                vector.drain()

                def wv(k):
                    return bass.AP(wts, k, [[NCO * 4, 128], [4, NCO]])

                vector.tensor_tensor(out=wv(0), in0=flat(u1), in1=flat(u3), op=AF.mult)
                vector.tensor_tensor(out=wv(1), in0=flat(u2), in1=flat(u3), op=AF.mult)
                vector.tensor_tensor(out=wv(2), in0=flat(u1), in1=flat(u4), op=AF.mult)
                vector.tensor_tensor(out=wv(3), in0=flat(u2), in1=flat(u4), op=AF.mult)
                vector.drain()

                def doblock(n):
                    pb = patches[n % 2]
                    vector.wait_ge(gat_sems[n % 2], 16 * (n // 2 + 1))
                    # prodW[p,(s,j,k,c)] = patch * left ; split per s (3 free dims max)
                    for s in range(S):
                        in0 = bass.AP(
                            pb,
                            s * JJ * ELEM,
                            [[NSLOT * ELEM, 128], [ELEM, JJ], [gC, 4], [1, gC]],
                        )
                        in1 = bass.AP(
                            lt_s,
                            n * JJ * gC,
                            [[NBLK * JJ * gC, 128], [gC, JJ], [0, 4], [1, gC]],
                        )
                        o = bass.AP(
                            prodw,
                            s * JJ * ELEM,
                            [[NSLOT * ELEM, 128], [ELEM, JJ], [gC, 4], [1, gC]],
                        )
                        mm = vector.tensor_tensor(out=o, in0=in0, in1=in1, op=AF.mult)
                    mm.then_inc(prod_sem, 1)
                    vector.drain()
                    # t[p, slot, k] = sum_c prodw
                    pin = bass.AP(
                        prodw, 0, [[NSLOT * ELEM, 128], [gC, NSLOT * 4], [1, gC]]
                    )
                    tv = bass.AP(tbuf, 0, [[NSLOT * 4, 128], [1, NSLOT * 4]])
                    with nc.allow_low_precision(reason="bf16 64-ch dot accumulate"):
                        vector.tensor_reduce(
                            out=tv, in_=pin, axis=mybir.AxisListType.X, op=AF.add
                        )
                    vector.drain()
                    # corr[p, slot] = sum_k t*w
                    if n >= 2:
                        vector.wait_ge(out_sems[n % 2], 16 * ((n - 2) // 2 + 1))
                    m = bass.AP(mbuf, 0, [[NSLOT * 4, 128], [1, NSLOT * 4]])
                    wv2 = bass.AP(
                        wts, n * NSLOT * 4, [[NCO * 4, 128], [1, NSLOT * 4]]
                    )
                    vector.tensor_tensor(out=m, in0=tv, in1=wv2, op=AF.mult)
                    vector.drain()
                    m3 = bass.AP(mbuf, 0, [[NSLOT * 4, 128], [4, NSLOT], [1, 4]])
                    cv = bass.AP(corrs[n % 2], 0, [[NSLOT, 128], [1, NSLOT]])
                    vector.tensor_reduce(
                        out=cv, in_=m3, axis=mybir.AxisListType.X, op=AF.add
                    ).then_inc(blend_sem, 1)
                    vector.drain()

                for n in range(NBLK):
                    doblock(n)

            @block.gpsimd
            def _(gpsimd):
                gpsimd.load_library(mlp_library)
                gpsimd.wait_ge(wrap_sem, 240)
                for n in range(NBLK):
                    if n >= 2:
                        gpsimd.wait_ge(prod_sem, n - 1)
                    pb = patches[n % 2]
                    dst = bass.AP(
                        pb, 0, [[NSLOT * ELEM, 128], [ELEM, NSLOT], [1, ELEM]]
                    )
                    idxs_ap = wrap[:, n * 360 : (n + 1) * 360]
                    gpsimd.dma_gather(
                        dst,
                        r4[:, :],
                        idxs_ap,
                        NIDX,
                        NIDX,
                        ELEM,
                        transpose=False,
                        single_packet=False,
                    ).then_inc(gat_sems[n % 2], 16)

            @block.scalar
            def _(scalar):
                for n in range(NBLK):
                    scalar.wait_ge(blend_sem, n + 1)
                    dst = out[n * 128 : (n + 1) * 128, :]
                    scalar.dma_start(dst, corrs[n % 2][:, :]).then_inc(
                        out_sems[n % 2], 16
                    )
                scalar.wait_ge(out_sems[0], 16 * (NBLK // 2))
                scalar.wait_ge(out_sems[1], 16 * (NBLK // 2))

    if not nc.is_finalized():
        nc.finalize()
    return nc


def _host_prep(left_feature, right_feature, flow, extra_offset):
    """Build per-core input dicts. Core ordering: core = b*4 + g."""
    lf = np.asarray(left_feature, np.float32)
    rf = np.asarray(right_feature, np.float32)
    fl = np.asarray(flow, np.float32)
    eo = np.asarray(extra_offset, np.float32)

    # sample (s, j, p) in block blk: pixel pi = j*128+p; (q, w) = divmod(pi, W)
    p_idx = np.arange(128)
    j_idx = np.arange(JJ)
    pi = j_idx[None, :] * 128 + p_idx[:, None]  # [128, 5]
    q = pi // W
    w = pi % W
    blk = np.arange(NBLK)
    h = blk[:, None, None] * 4 + q[None]  # [20, 128, 5]
    wfull = np.broadcast_to(w[None], (NBLK, 128, JJ))

    offx = np.arange(S, dtype=np.float32) - 4.0

    in_maps = []
    for b in range(B):
        fx = fl[b, 0][h, wfull]  # [20,128,5]
        fy = fl[b, 1][h, wfull]
        cbx = wfull.astype(np.float32) + fx + PADDING
        cby = h.astype(np.float32) + fy + PADDING
        cb_np = np.ascontiguousarray(
            np.stack([cbx, cby], -1).transpose(1, 0, 2, 3).reshape(128, -1)
        )

        eo_b = eo[b].reshape(S, 2, H, W)
        exx = eo_b[:, 0][:, h, wfull] + offx[:, None, None, None]  # [S,20,128,5]
        exy = eo_b[:, 1][:, h, wfull]
        ex_np = np.ascontiguousarray(
            np.stack([exx, exy], -1)  # [S,20,128,5,2]
            .transpose(2, 1, 0, 3, 4)  # [128,20,S,5,2]
            .reshape(128, -1)
        )

        for g in range(G):
            csl = slice(g * gC, (g + 1) * gC)
            # lt[p, blk, j, c] = left[c, blk*4 + pi//W, pi%W] / gC, pi=j*128+p
            lslice = lf[b, csl] / gC  # [64, H, W]
            l4 = lslice[:, h, wfull]  # [64, 20, 128, 5]
            lt_np = np.ascontiguousarray(
                l4.transpose(2, 1, 3, 0).reshape(128, -1).astype(ml_dtypes.bfloat16)
            )

            # r4 patch table
            rp = np.zeros((TAB_H + 1, TAB_W + 1, gC), np.float32)
            rp[PADDING : PADDING + H, PADDING : PADDING + W] = rf[b, csl].transpose(
                1, 2, 0
            )
            r4_np = np.ascontiguousarray(
                np.stack(
                    [
                        rp[0:TAB_H, 0:TAB_W],
                        rp[0:TAB_H, 1 : TAB_W + 1],
                        rp[1 : TAB_H + 1, 0:TAB_W],
                        rp[1 : TAB_H + 1, 1 : TAB_W + 1],
                    ],
                    axis=2,
                )
                .reshape(NROWS, ELEM)
                .astype(ml_dtypes.bfloat16)
            )

            in_maps.append(
                {
                    "r4": r4_np,
                    "lt": lt_np,
                    "cb": cb_np,
                    "ex": ex_np,
                }
            )
    return in_maps


def kernel(**inputs):
    if "nc" not in _graph_cache:
        _graph_cache["nc"] = _build_graph()
    nc = _graph_cache["nc"]

    in_maps = _host_prep(
        inputs["left_feature"],
        inputs["right_feature"],
        inputs["flow"],
        inputs["extra_offset"],
    )
    res = run_bass_kernel_spmd(nc, in_maps, core_ids=list(range(8)))
    _graph_cache["last_res"] = res
    outs = [r["out"] for r in res.results]

    full = np.zeros((B, G * S, H, W), np.float32)
    for core in range(8):
        b, g = divmod(core, G)
        o = np.asarray(outs[core], np.float32).reshape(NBLK, 128, S, JJ)
        # [blk, p, s, j] -> [blk, s, (j,p)=pix] -> [blk, s, q, w]
        o = o.transpose(0, 2, 3, 1).reshape(NBLK, S, 4, W)
        for s in range(S):
            full[b, g * S + s] = o[:, s].reshape(H, W)
    return full
